# revision 14
# baseline (speedup 1.0000x reference)
"""BitLinear (absmean ternary W x absmax int8 activations) on 8 TRN2 cores.

out[b,s,o] = sum_k x_q[b,s,k] * w_q[o,k]
  w_q = clip(round(W/gw), -1, 1) * gw,        gw = mean(|W|) + 1e-5   (global)
  x_q = clip(round(127*x/gx), -128, 127)*gx/127,  gx = max_k(|x|) + 1e-5 (per row)

Strategy (tensor parallel): shard W rows (out features, 11008 = 8*1376)
across 8 cores, replicate x. The quantized values are small integers
(x: |.|<=127, W: {-1,0,1}) that are exact in bf16, so the matmul runs on
the PE at bf16 rate with exact integer accumulation in fp32 PSUM; outputs
get one final scale of gx[row]*gw/127. gw needs a scalar AllReduce of
per-shard |W| sums.

Rounding to integers uses the fp32 magic-number trick (+1.5*2^23, -1.5*2^23)
which is round-to-nearest-even, matching jnp.round.

Transposes (both matmul operands need K on partitions) go through the DMA
xbar (dma_start_transpose, bf16): in_=[R, 4096] -> out=[128, 32, R] gives
out[p, a, b] = in[b, 128*a + p], i.e. slice [:, a, :] is the a-th K-slice
in standard [k_partition, free] layout. Note: the xbar serializes against
an in-flight collective, so no transposes are issued while the AllReduce
is outstanding.

A dummy AllReduce is issued at kernel start so the NRT collectives-init
barrier (which waits for all 8 cores and absorbs launch skew) overlaps
the |W| pass instead of sitting on the critical path.
"""

import numpy as np
from contextlib import ExitStack

import concourse.bass as bass
import concourse.mybir as mybir
import concourse.tile as tile
from concourse import bacc
from concourse.bass_utils import run_bass_kernel_spmd

P = 128
M_TOT = 16384            # 4*4096 flattened x rows
K = 4096
KS = K // P              # 32 k-slices
N_REAL = 11008
N_CORES = 8
N_PER = 1376             # per-core out features (exact: 8*1376 = 11008)
MT = M_TOT // P          # 128 m-tiles
CHUNKS = ((0, 512), (512, 512), (1024, 352))
W_TILES = 11             # 10 full 128-row tiles + one 96-row tile
MAGIC = 12582912.0       # 1.5 * 2**23: fp32 round-to-nearest-int trick
EPS = 1e-5
STAGE = 2                # x m-tiles staged during W prep
F32 = mybir.dt.float32
BF16 = mybir.dt.bfloat16
X_AX = mybir.AxisListType.X
ALU = mybir.AluOpType
ACTF = mybir.ActivationFunctionType

_CACHE = {}


def _build() -> bass.Bass:
    nc = bacc.Bacc("TRN2", target_bir_lowering=False, debug=False,
                   num_devices=N_CORES)
    x_d = nc.dram_tensor("x", [M_TOT, K], F32, kind="ExternalInput").ap()
    w_d = nc.dram_tensor("w", [N_PER, K], F32, kind="ExternalInput").ap()
    o_d = nc.dram_tensor("out", [M_TOT, N_PER], F32, kind="ExternalOutput").ap()

    with ExitStack() as ctx:
        tc = ctx.enter_context(tile.TileContext(nc))
        ld = ctx.enter_context(tc.tile_pool(name="ld", bufs=3))      # [128,4096] f32 loads
        qp = ctx.enter_context(tc.tile_pool(name="qp", bufs=2))      # [128,4096] bf16
        tp = ctx.enter_context(tc.tile_pool(name="tp", bufs=3))      # [128,32,128] bf16 xT
        wtp = ctx.enter_context(tc.tile_pool(name="wtp", bufs=1))    # persistent W^T chunks
        op = ctx.enter_context(tc.tile_pool(name="op", bufs=2))      # [128,1376] f32 out stage
        sc = ctx.enter_context(tc.tile_pool(name="sc", bufs=2))      # per-m-tile scalars
        pc = ctx.enter_context(tc.tile_pool(name="pc", bufs=1))      # persistent small consts
        acc = ctx.enter_context(tc.tile_pool(name="acc", bufs=2, space="PSUM"))
        dp = ctx.enter_context(tc.tile_pool(name="dp", bufs=1, space="DRAM"))

        wt_chunks = [wtp.tile([P, KS, w], BF16, tag=f"wt{c}", name=f"wt{c}")
                     for c, (_, w) in enumerate(CHUNKS)]

        ones_col = pc.tile([P, 1], F32, tag="ones_col")
        nc.vector.memset(ones_col[:], 1.0)
        ones_row = pc.tile([1, P], F32, tag="ones_row")
        nc.vector.memset(ones_row[:], 1.0)
        magic = pc.tile([P, 1], F32, tag="magic")
        nc.vector.memset(magic[:], MAGIC)

        # ---- dummy collective: absorb the collectives-init barrier early ----
        dum_sb = pc.tile([1, 1], F32, tag="dum_sb")
        nc.vector.memset(dum_sb[:], 0.0)
        dum_in = dp.tile([1, 1], F32, tag="dum_in", name="dum_in")
        dum_out = dp.tile([1, 1], F32, tag="dum_out", name="dum_out",
                          addr_space="Shared")
        nc.scalar.dma_start(dum_in[:], dum_sb[:])
        nc.gpsimd.collective_compute(
            "AllReduce", ALU.add, replica_groups=[list(range(N_CORES))],
            ins=[dum_in[:]], outs=[dum_out[:]])

        def w_rows(r):
            return 96 if r == W_TILES - 1 else P

        # ---- W pass 1: |W| partial sums ----
        pabs = pc.tile([P, 16], F32, tag="pabs")
        nc.vector.memset(pabs[:], 0.0)
        for r in range(W_TILES):
            rows = w_rows(r)
            wt = ld.tile([P, K], F32, tag="ld", name="wld")
            nc.sync.dma_start(wt[:rows, :], w_d[r * P:r * P + rows, :])
            nc.vector.tensor_reduce(pabs[:rows, r:r + 1], wt[:rows, :],
                                    axis=X_AX, op=ALU.add,
                                    apply_absolute_value=True)
        rowsum = pc.tile([P, 1], F32, tag="rowsum")
        nc.vector.tensor_reduce(rowsum[:], pabs[:, 0:W_TILES], axis=X_AX,
                                op=ALU.add)
        tot_ps = acc.tile([1, 1], F32, tag="misc", name="tot_ps")
        nc.tensor.matmul(tot_ps[:], lhsT=rowsum[:], rhs=ones_col[:],
                         start=True, stop=True)
        tot_sb = pc.tile([1, 1], F32, tag="tot_sb")
        nc.vector.tensor_copy(tot_sb[:], tot_ps[:])
        cc_in = dp.tile([1, 1], F32, tag="cc_in", name="cc_in")
        cc_out = dp.tile([1, 1], F32, tag="cc_out", name="cc_out",
                         addr_space="Shared")
        nc.scalar.dma_start(cc_in[:], tot_sb[:])
        nc.gpsimd.collective_compute(
            "AllReduce", ALU.add, replica_groups=[list(range(N_CORES))],
            ins=[cc_in[:]], outs=[cc_out[:]])

        # ---- x pipeline pieces ----
        def x_load(mt):
            xt = ld.tile([P, K], F32, tag="ld", name="xld")
            nc.sync.dma_start(xt[:], x_d[mt * P:(mt + 1) * P, :])
            return xt

        def x_quant(xt):
            gmax = sc.tile([P, 1], F32, tag="gmax", name="gmax")
            nc.vector.tensor_reduce(gmax[:], xt[:], axis=X_AX, op=ALU.max,
                                    apply_absolute_value=True)
            # gp = (max|x| + eps)/127 ; s = 127/gx
            gp = sc.tile([P, 1], F32, tag="gp", name="gp", bufs=STAGE + 2)
            nc.vector.tensor_scalar(out=gp[:], in0=gmax[:], scalar1=EPS,
                                    scalar2=1.0 / 127.0, op0=ALU.add,
                                    op1=ALU.mult)
            s = sc.tile([P, 1], F32, tag="s", name="s")
            nc.vector.reciprocal(s[:], gp[:])
            nc.scalar.activation(xt[:], xt[:], ACTF.Identity,
                                 bias=magic[:], scale=s[:])
            xq = qp.tile([P, K], BF16, tag="qp", name="xq")
            nc.vector.tensor_scalar_sub(xq[:], xt[:], MAGIC)
            return xq, gp

        def x_transpose(xq):
            xT = tp.tile([P, KS, P], BF16, tag="tp", name="xT")
            nc.sync.dma_start_transpose(xT[:], xq[:, :])
            return xT

        # stage a couple of m-tiles' quantized data during the collective
        staged_q = []
        for mt in range(STAGE):
            xt = x_load(mt)
            staged_q.append(x_quant(xt))

        # ---- gw math (after collective) ----
        gtot = pc.tile([1, 1], F32, tag="gtot")
        nc.scalar.dma_start(gtot[:], cc_out[:])
        gw = pc.tile([1, 1], F32, tag="gw")
        nc.vector.tensor_scalar(out=gw[:], in0=gtot[:],
                                scalar1=1.0 / (N_REAL * K), scalar2=EPS,
                                op0=ALU.mult, op1=ALU.add)
        bc_ps = acc.tile([P, 1], F32, tag="misc", name="bc_ps")
        nc.tensor.matmul(bc_ps[:], lhsT=ones_row[:], rhs=gw[:],
                         start=True, stop=True)
        gw_bc = pc.tile([P, 1], F32, tag="gw_bc")
        nc.vector.tensor_copy(gw_bc[:], bc_ps[:])
        gw_inv = pc.tile([P, 1], F32, tag="gw_inv")
        nc.vector.reciprocal(gw_inv[:], gw_bc[:])

        # ---- W pass 2: ternarize (exact in bf16) + xbar transpose ----
        def w_load(r):
            rows = w_rows(r)
            wt = ld.tile([P, K], F32, tag="ld", name="wld2")
            nc.sync.dma_start(wt[:rows, :], w_d[r * P:r * P + rows, :])
            return wt

        # Staged x transposes first: they free the qp slots the W quant needs,
        # and the xbar is serialized against the in-flight AllReduce anyway.
        staged_T = [x_transpose(xq) for xq, _ in staged_q]

        # W transposes are deferred 2 iterations behind their producing tile so
        # a transpose at the sync-ring head never blocks on an unfinished clip
        # (which would stall the loads queued behind it).
        wqs = {}

        def emit_wT(r):
            wq, rows = wqs.pop(r)
            c, col = divmod(r, 4)
            nc.sync.dma_start_transpose(
                wt_chunks[c][:, :, col * P:col * P + rows], wq[:rows, :])

        wts = {0: w_load(0), 1: w_load(1)}
        for r in range(W_TILES):
            if r >= 2:
                emit_wT(r - 2)
            if r + 2 < W_TILES:
                wts[r + 2] = w_load(r + 2)
            wt = wts.pop(r)
            rows = w_rows(r)
            # round(W/gw): W*(1/gw) + MAGIC on ACT, then -MAGIC & clip on DVE
            nc.scalar.activation(wt[:rows, :], wt[:rows, :], ACTF.Identity,
                                 bias=magic[:rows], scale=gw_inv[:rows])
            nc.vector.tensor_scalar(out=wt[:rows, :], in0=wt[:rows, :],
                                    scalar1=MAGIC, scalar2=1.0,
                                    op0=ALU.subtract, op1=ALU.min)
            wq = qp.tile([P, K], BF16, tag="qp", name="wq")
            nc.vector.tensor_scalar_max(wq[:rows, :], wt[:rows, :], -1.0)
            wqs[r] = (wq, rows)
        for r in (W_TILES - 2, W_TILES - 1):
            emit_wT(r)

        # ---- per-chunk matmul + eviction ----
        def mm_chunk(c, xT, a_out, s2, osb):
            off, w = CHUNKS[c]
            a = acc.tile([P, w], F32, tag=f"acc{c}", name=f"acc{c}")
            for ks in range(KS):
                nc.tensor.matmul(a[:], lhsT=xT[:, ks, :],
                                 rhs=wt_chunks[c][:, ks, :],
                                 start=(ks == 0), stop=(ks == KS - 1))
            nc.scalar.activation(osb[:, off:off + w], a[:], ACTF.Copy,
                                 scale=s2[:])

        def make_s2(gp):
            s2 = sc.tile([P, 1], F32, tag="s2", name="s2", bufs=STAGE + 2)
            nc.vector.tensor_mul(s2[:], gp[:], gw_bc[:])
            return s2

        # staged m-tiles: chunk-major so the PE can start on chunk 0 while
        # later W chunks are still being transposed
        staged_s2 = [make_s2(gp) for _, gp in staged_q]
        staged_osb = [op.tile([P, N_PER], F32, tag="op", name="osb")
                      for _ in range(STAGE)]
        for c in range(len(CHUNKS)):
            for i in range(STAGE):
                mm_chunk(c, staged_T[i], None, staged_s2[i], staged_osb[i])
        for i in range(STAGE):
            nc.sync.dma_start(o_d[i * P:(i + 1) * P, :], staged_osb[i][:])

        # ---- main loop with one-tile load prefetch ----
        pend = x_load(STAGE)
        for mt in range(STAGE, MT):
            xt = pend
            if mt + 1 < MT:
                pend = x_load(mt + 1)
            xq, gp = x_quant(xt)
            xT = x_transpose(xq)
            s2 = make_s2(gp)
            osb = op.tile([P, N_PER], F32, tag="op", name="osb")
            for c in range(len(CHUNKS)):
                mm_chunk(c, xT, None, s2, osb)
            nc.sync.dma_start(o_d[mt * P:(mt + 1) * P, :], osb[:])
    nc.compile()
    return nc


def _get_nc() -> bass.Bass:
    if "nc" not in _CACHE:
        _CACHE["nc"] = _build()
    return _CACHE["nc"]


def _shard_inputs(x: np.ndarray, weight: np.ndarray):
    x2 = np.ascontiguousarray(x.reshape(M_TOT, K).astype(np.float32, copy=False))
    w = weight.astype(np.float32, copy=False)
    return [{"x": x2, "w": np.ascontiguousarray(w[i * N_PER:(i + 1) * N_PER])}
            for i in range(N_CORES)]


def _gather(results) -> np.ndarray:
    full = np.concatenate([results[i]["out"] for i in range(N_CORES)], axis=1)
    return np.ascontiguousarray(full).reshape(4, 4096, N_REAL)


def run(x: np.ndarray, weight: np.ndarray, **spmd_kwargs):
    nc = _get_nc()
    in_maps = _shard_inputs(x, weight)
    br = run_bass_kernel_spmd(nc, in_maps, list(range(N_CORES)), **spmd_kwargs)
    return _gather(br.results), br


def kernel(x: np.ndarray, weight: np.ndarray) -> np.ndarray:
    out, _ = run(x, weight)
    return out


# revision 17
# speedup vs baseline: 1.2056x; 1.2056x over previous
"""BitLinear (absmean ternary W x absmax int8 activations) on 8 TRN2 cores.

out[b,s,o] = sum_k x_q[b,s,k] * w_q[o,k]
  w_q = clip(round(W/gw), -1, 1) * gw,        gw = mean(|W|) + 1e-5   (global)
  x_q = clip(round(127*x/gx), -128, 127)*gx/127,  gx = max_k(|x|) + 1e-5 (per row)

Strategy (tensor parallel): shard W rows (out features, 11008 = 8*1376)
across 8 cores, replicate x. The quantized values are small integers
(x: |.|<=127, W: {-1,0,1}) that are exact in bf16, so the matmul runs on
the PE at bf16 rate with exact integer accumulation in fp32 PSUM; outputs
get one final scale of gx[row]*gw/127. gw needs a scalar AllReduce of
per-shard |W| sums.

Rounding to integers uses the fp32 magic-number trick (+1.5*2^23, -1.5*2^23)
which is round-to-nearest-even, matching jnp.round.

Transposes (both matmul operands need K on partitions) go through the DMA
xbar (dma_start_transpose, bf16): in_=[R, 4096] -> out=[128, 32, R] gives
out[p, a, b] = in[b, 128*a + p], i.e. slice [:, a, :] is the a-th K-slice
in standard [k_partition, free] layout. Note: the xbar serializes against
an in-flight collective, so no transposes are issued while the AllReduce
is outstanding.

A dummy AllReduce is issued at kernel start so the NRT collectives-init
barrier (which waits for all 8 cores and absorbs launch skew) overlaps
the |W| pass instead of sitting on the critical path.
"""

import numpy as np
from contextlib import ExitStack

import concourse.bass as bass
import concourse.mybir as mybir
import concourse.tile as tile
from concourse import bacc
from concourse.bass_utils import run_bass_kernel_spmd

P = 128
M_TOT = 16384            # 4*4096 flattened x rows
K = 4096
KS = K // P              # 32 k-slices
N_REAL = 11008
N_CORES = 8
N_PER = 1376             # per-core out features (exact: 8*1376 = 11008)
MT = M_TOT // P          # 128 m-tiles
CHUNKS = ((0, 512), (512, 512), (1024, 352))
W_TILES = 11             # 10 full 128-row tiles + one 96-row tile
MAGIC = 12582912.0       # 1.5 * 2**23: fp32 round-to-nearest-int trick
EPS = 1e-5
STAGE = 2                # x m-tiles staged during W prep
F32 = mybir.dt.float32
BF16 = mybir.dt.bfloat16
X_AX = mybir.AxisListType.X
ALU = mybir.AluOpType
ACTF = mybir.ActivationFunctionType

_CACHE = {}


def _build() -> bass.Bass:
    nc = bacc.Bacc("TRN2", target_bir_lowering=False, debug=False,
                   num_devices=N_CORES)
    x_d = nc.dram_tensor("x", [M_TOT, K], F32, kind="ExternalInput").ap()
    w_d = nc.dram_tensor("w", [N_PER, K], F32, kind="ExternalInput").ap()
    o_d = nc.dram_tensor("out", [M_TOT, N_PER], F32, kind="ExternalOutput").ap()

    with ExitStack() as ctx:
        tc = ctx.enter_context(tile.TileContext(nc))
        ld = ctx.enter_context(tc.tile_pool(name="ld", bufs=3))      # [128,4096] f32 loads
        qp = ctx.enter_context(tc.tile_pool(name="qp", bufs=2))      # [128,4096] bf16
        tp = ctx.enter_context(tc.tile_pool(name="tp", bufs=3))      # [128,32,128] bf16 xT
        wtp = ctx.enter_context(tc.tile_pool(name="wtp", bufs=1))    # persistent W^T chunks
        op = ctx.enter_context(tc.tile_pool(name="op", bufs=2))      # [128,1376] f32 out stage
        sc = ctx.enter_context(tc.tile_pool(name="sc", bufs=2))      # per-m-tile scalars
        pc = ctx.enter_context(tc.tile_pool(name="pc", bufs=1))      # persistent small consts
        acc = ctx.enter_context(tc.tile_pool(name="acc", bufs=2, space="PSUM"))
        dp = ctx.enter_context(tc.tile_pool(name="dp", bufs=1, space="DRAM"))

        # W row-tiles 0-4 stay resident across the collective in the same SBUF
        # slots the W^T chunks will later occupy (shared pool tags -> the
        # chunk writes get WAR deps on the cached reads). Saves reloading
        # 10 MB of W after gw arrives and keeps the post-collective
        # transpose burst free of copy<->xbar transitions.
        wc01 = wtp.tile([P, 2, K], F32, tag="wt0", name="wc01")
        wc23 = wtp.tile([P, 2, K], F32, tag="wt1", name="wc23")
        wc4 = wtp.tile([P, K], F32, tag="wt2", name="wc4")
        W_CACHED = 5

        def wcache(r):
            return (wc01[:, 0, :], wc01[:, 1, :], wc23[:, 0, :],
                    wc23[:, 1, :], wc4[:, :])[r]

        wt_chunks = [wtp.tile([P, KS, w], BF16, tag=f"wt{c}", name=f"wt{c}")
                     for c, (_, w) in enumerate(CHUNKS)]

        ones_col = pc.tile([P, 1], F32, tag="ones_col")
        nc.vector.memset(ones_col[:], 1.0)
        ones_row = pc.tile([1, P], F32, tag="ones_row")
        nc.vector.memset(ones_row[:], 1.0)
        magic = pc.tile([P, 1], F32, tag="magic")
        nc.vector.memset(magic[:], MAGIC)

        # ---- dummy collective: absorb the collectives-init barrier early ----
        dum_sb = pc.tile([1, 1], F32, tag="dum_sb")
        nc.vector.memset(dum_sb[:], 0.0)
        dum_in = dp.tile([1, 1], F32, tag="dum_in", name="dum_in")
        dum_out = dp.tile([1, 1], F32, tag="dum_out", name="dum_out",
                          addr_space="Shared")
        nc.scalar.dma_start(dum_in[:], dum_sb[:])
        nc.gpsimd.collective_compute(
            "AllReduce", ALU.add, replica_groups=[list(range(N_CORES))],
            ins=[dum_in[:]], outs=[dum_out[:]])

        def w_rows(r):
            return 96 if r == W_TILES - 1 else P

        # ---- W pass 1: |W| partial sums (tiles 0-4 land in the cache) ----
        pabs = pc.tile([P, 16], F32, tag="pabs")
        nc.vector.memset(pabs[:], 0.0)
        for r in range(W_TILES):
            rows = w_rows(r)
            if r < W_CACHED:
                wt = wcache(r)
            else:
                wt = ld.tile([P, K], F32, tag="ld", name="wld")[:, :]
            nc.sync.dma_start(wt[:rows, :], w_d[r * P:r * P + rows, :])
            nc.vector.tensor_reduce(pabs[:rows, r:r + 1], wt[:rows, :],
                                    axis=X_AX, op=ALU.add,
                                    apply_absolute_value=True)
        rowsum = pc.tile([P, 1], F32, tag="rowsum")
        nc.vector.tensor_reduce(rowsum[:], pabs[:, 0:W_TILES], axis=X_AX,
                                op=ALU.add)
        tot_ps = acc.tile([1, 1], F32, tag="misc", name="tot_ps")
        nc.tensor.matmul(tot_ps[:], lhsT=rowsum[:], rhs=ones_col[:],
                         start=True, stop=True)
        tot_sb = pc.tile([1, 1], F32, tag="tot_sb")
        nc.vector.tensor_copy(tot_sb[:], tot_ps[:])
        cc_in = dp.tile([1, 1], F32, tag="cc_in", name="cc_in")
        cc_out = dp.tile([1, 1], F32, tag="cc_out", name="cc_out",
                         addr_space="Shared")
        nc.scalar.dma_start(cc_in[:], tot_sb[:])
        nc.gpsimd.collective_compute(
            "AllReduce", ALU.add, replica_groups=[list(range(N_CORES))],
            ins=[cc_in[:]], outs=[cc_out[:]])

        # ---- x pipeline pieces ----
        def x_load(mt):
            xt = ld.tile([P, K], F32, tag="ld", name="xld")
            nc.sync.dma_start(xt[:], x_d[mt * P:(mt + 1) * P, :])
            return xt

        def x_quant(xt):
            gmax = sc.tile([P, 1], F32, tag="gmax", name="gmax")
            nc.vector.tensor_reduce(gmax[:], xt[:], axis=X_AX, op=ALU.max,
                                    apply_absolute_value=True)
            # gp = (max|x| + eps)/127 ; s = 127/gx
            gp = sc.tile([P, 1], F32, tag="gp", name="gp", bufs=STAGE + 2)
            nc.vector.tensor_scalar(out=gp[:], in0=gmax[:], scalar1=EPS,
                                    scalar2=1.0 / 127.0, op0=ALU.add,
                                    op1=ALU.mult)
            s = sc.tile([P, 1], F32, tag="s", name="s")
            nc.vector.reciprocal(s[:], gp[:])
            nc.scalar.activation(xt[:], xt[:], ACTF.Identity,
                                 bias=magic[:], scale=s[:])
            xq = qp.tile([P, K], BF16, tag="qp", name="xq")
            nc.vector.tensor_scalar_sub(xq[:], xt[:], MAGIC)
            return xq, gp

        def x_transpose(xq):
            xT = tp.tile([P, KS, P], BF16, tag="tp", name="xT")
            nc.sync.dma_start_transpose(xT[:], xq[:, :])
            return xT

        # stage a couple of m-tiles' quantized data during the collective
        staged_q = []
        for mt in range(STAGE):
            xt = x_load(mt)
            staged_q.append(x_quant(xt))

        # ---- gw math (after collective) ----
        gtot = pc.tile([1, 1], F32, tag="gtot")
        nc.scalar.dma_start(gtot[:], cc_out[:])
        gw = pc.tile([1, 1], F32, tag="gw")
        nc.vector.tensor_scalar(out=gw[:], in0=gtot[:],
                                scalar1=1.0 / (N_REAL * K), scalar2=EPS,
                                op0=ALU.mult, op1=ALU.add)
        bc_ps = acc.tile([P, 1], F32, tag="misc", name="bc_ps")
        nc.tensor.matmul(bc_ps[:], lhsT=ones_row[:], rhs=gw[:],
                         start=True, stop=True)
        gw_bc = pc.tile([P, 1], F32, tag="gw_bc")
        nc.vector.tensor_copy(gw_bc[:], bc_ps[:])
        gw_inv = pc.tile([P, 1], F32, tag="gw_inv")
        nc.vector.reciprocal(gw_inv[:], gw_bc[:])

        # ---- W pass 2: ternarize (exact in bf16) + xbar transpose ----
        def w_load(r):
            rows = w_rows(r)
            wt = ld.tile([P, K], F32, tag="ld", name="wld2")
            nc.sync.dma_start(wt[:rows, :], w_d[r * P:r * P + rows, :])
            return wt

        # Staged x transposes first: they free the qp slots the W quant needs,
        # and the xbar is serialized against the in-flight AllReduce anyway.
        staged_T = [x_transpose(xq) for xq, _ in staged_q]

        def w_quant_T(wt, r):
            rows = w_rows(r)
            # round(W/gw): W*(1/gw) + MAGIC on ACT, then -MAGIC & clip on DVE
            nc.scalar.activation(wt[:rows, :], wt[:rows, :], ACTF.Identity,
                                 bias=magic[:rows], scale=gw_inv[:rows])
            nc.vector.tensor_scalar(out=wt[:rows, :], in0=wt[:rows, :],
                                    scalar1=MAGIC, scalar2=1.0,
                                    op0=ALU.subtract, op1=ALU.min)
            wq = qp.tile([P, K], BF16, tag="qp", name="wq")
            nc.vector.tensor_scalar_max(wq[:rows, :], wt[:rows, :], -1.0)
            c, col = divmod(r, 4)
            nc.sync.dma_start_transpose(
                wt_chunks[c][:, :, col * P:col * P + rows], wq[:rows, :])

        # cached tiles: no loads interleave, so the xbar stays in transpose
        # mode for the whole burst
        for r in range(W_CACHED):
            w_quant_T(wcache(r), r)
        # remaining tiles stream from HBM
        wts = {W_CACHED: w_load(W_CACHED), W_CACHED + 1: w_load(W_CACHED + 1)}
        for r in range(W_CACHED, W_TILES):
            if r + 2 < W_TILES:
                wts[r + 2] = w_load(r + 2)
            w_quant_T(wts.pop(r)[:, :], r)

        # ---- per-chunk matmul + eviction ----
        def mm_chunk(c, xT, a_out, s2, osb):
            off, w = CHUNKS[c]
            a = acc.tile([P, w], F32, tag=f"acc{c}", name=f"acc{c}")
            for ks in range(KS):
                nc.tensor.matmul(a[:], lhsT=xT[:, ks, :],
                                 rhs=wt_chunks[c][:, ks, :],
                                 start=(ks == 0), stop=(ks == KS - 1))
            nc.scalar.activation(osb[:, off:off + w], a[:], ACTF.Copy,
                                 scale=s2[:])

        def make_s2(gp):
            s2 = sc.tile([P, 1], F32, tag="s2", name="s2", bufs=STAGE + 2)
            nc.vector.tensor_mul(s2[:], gp[:], gw_bc[:])
            return s2

        # staged m-tiles: chunk-major so the PE can start on chunk 0 while
        # later W chunks are still being transposed
        staged_s2 = [make_s2(gp) for _, gp in staged_q]
        staged_osb = [op.tile([P, N_PER], F32, tag="op", name="osb")
                      for _ in range(STAGE)]
        for c in range(len(CHUNKS)):
            for i in range(STAGE):
                mm_chunk(c, staged_T[i], None, staged_s2[i], staged_osb[i])
        for i in range(STAGE):
            nc.sync.dma_start(o_d[i * P:(i + 1) * P, :], staged_osb[i][:])

        # ---- main loop with one-tile load prefetch ----
        pend = x_load(STAGE)
        for mt in range(STAGE, MT):
            xt = pend
            if mt + 1 < MT:
                pend = x_load(mt + 1)
            xq, gp = x_quant(xt)
            xT = x_transpose(xq)
            s2 = make_s2(gp)
            osb = op.tile([P, N_PER], F32, tag="op", name="osb")
            for c in range(len(CHUNKS)):
                mm_chunk(c, xT, None, s2, osb)
            nc.sync.dma_start(o_d[mt * P:(mt + 1) * P, :], osb[:])
    nc.compile()
    return nc


def _get_nc() -> bass.Bass:
    if "nc" not in _CACHE:
        _CACHE["nc"] = _build()
    return _CACHE["nc"]


def _shard_inputs(x: np.ndarray, weight: np.ndarray):
    x2 = np.ascontiguousarray(x.reshape(M_TOT, K).astype(np.float32, copy=False))
    w = weight.astype(np.float32, copy=False)
    return [{"x": x2, "w": np.ascontiguousarray(w[i * N_PER:(i + 1) * N_PER])}
            for i in range(N_CORES)]


def _gather(results) -> np.ndarray:
    full = np.concatenate([results[i]["out"] for i in range(N_CORES)], axis=1)
    return np.ascontiguousarray(full).reshape(4, 4096, N_REAL)


def run(x: np.ndarray, weight: np.ndarray, **spmd_kwargs):
    nc = _get_nc()
    in_maps = _shard_inputs(x, weight)
    br = run_bass_kernel_spmd(nc, in_maps, list(range(N_CORES)), **spmd_kwargs)
    return _gather(br.results), br


def kernel(x: np.ndarray, weight: np.ndarray) -> np.ndarray:
    out, _ = run(x, weight)
    return out
